# revision 32
# baseline (speedup 1.0000x reference)
"""Trainium2 Bass kernel for nn_LlamaAttention_48816598286577.

Llama attention with block-streaming sparse mask (sink=1 block, local
window=8 blocks, BLOCK=128), B=1 S=2048 H=4096, 32 q heads / 8 kv heads,
head_dim 128, non-interleaved RoPE.

Sharding: tensor-parallel over heads across 8 cores (4 q heads + 1 kv
head per core). All compute in bf16 (PSUM accumulates f32): full PE rate
at any matmul width, half the DMA/collective traffic of f32.

Structure:
- Host pre-swizzles hid/weights chunk-major so every DMA has >=1KB
  contiguous lines per partition; weight and quarter-0 hid DMAs are
  interleaved so the first projection matmul issues within ~10us.
- Two tiny warm-up AllGathers absorb the ~100us CC-stream barrier under
  phase 1.
- V natural blocks carry an appended ones-column and the PV matmul uses
  e_t (exp scores) as stationary -> PSUM holds natural-layout
  [q, d | rowsum]; softmax denominator falls out of the matmul chain and
  normalization is a per-partition tensor_scalar_mul.
- o_proj for chunk i-3 is interleaved between the attention heads of
  chunk i (ag_sb DMAs prefetched one head earlier) so the PE never
  stalls on the AllGather or on ACT exp lag.
- Phase-2 SBUF pools are allocated below phase-1 pools (not reused) so
  the phase boundary carries no write-after-read waits.
"""

import functools
import numpy as np

import concourse.bass as bass
import concourse.mybir as mybir
import concourse.tile as tile
from concourse import bacc
from concourse.bass_utils import run_bass_kernel_spmd

# problem constants (hardcoded per contract)
B, S, H = 1, 2048, 4096
NQ, NKV, HD = 32, 8, 128
BLOCK = 128
NBLK = S // BLOCK          # 16
SINK_BLOCKS = 1
LOCAL_BLOCKS = 8
ROPE_BASE = 10000.0
N_CORES = 8
HQ = NQ // N_CORES         # 4 q heads per core
DQ = HQ * HD               # 512 q columns per core
SCALE = 1.0 / float(np.sqrt(HD))

KC = H // 128              # 32 contraction chunks for projections
NQUART = 4                 # S split into 4 quarters of 512 for projections
QW = S // NQUART           # 512
NPAIR = NBLK // 2          # 8 query pairs of 256
NPRE = 24                  # quarter-0 hid chunks preloaded with the weights

F32 = mybir.dt.float32
BF16 = mybir.dt.bfloat16

VB = 129                   # v-block stride in vNat (128 v cols + ones col)
DEPTH = 3                  # o_proj pipeline depth in chunks


def _pair_blocks(i: int):
    """Key blocks for query pair i with per-block subblock coverage.

    Returns list of (j, left, right): left/right = whether q-block 2i /
    2i+1 attends to key block j (causal + sink-or-local, block level).
    """
    out = []
    for j in range(2 * i + 2):
        left = j <= 2 * i and (2 * i - j < LOCAL_BLOCKS or j < SINK_BLOCKS)
        right = j <= 2 * i + 1 and (2 * i + 1 - j < LOCAL_BLOCKS or j < SINK_BLOCKS)
        if left or right:
            out.append((j, left, right))
    return out


def build_nc():
    nc = bacc.Bacc(
        "TRN2", target_bir_lowering=False, debug=False, num_devices=N_CORES
    )
    # chunk-major swizzled inputs (see _run): index [p, c*W + x] holds
    # original [c*128 + p, x].
    hid_sw = nc.dram_tensor("hid_sw", [128, KC * S], BF16, kind="ExternalInput").ap()
    wq_sw = nc.dram_tensor("wq_sw", [128, KC * DQ], BF16, kind="ExternalInput").ap()
    wk_sw = nc.dram_tensor("wk_sw", [128, KC * HD], BF16, kind="ExternalInput").ap()
    wv_sw = nc.dram_tensor("wv_sw", [128, KC * HD], BF16, kind="ExternalInput").ap()
    wo_sw = nc.dram_tensor("wo_sw", [128, KC * DQ], BF16, kind="ExternalInput").ap()
    cosF = nc.dram_tensor("cosF", [128, S], F32, kind="ExternalInput").ap()
    sinS = nc.dram_tensor("sinS", [128, S], F32, kind="ExternalInput").ap()
    tri = nc.dram_tensor("tri", [128, 128], BF16, kind="ExternalInput").ap()
    eye = nc.dram_tensor("eye", [128, 128], BF16, kind="ExternalInput").ap()
    out = nc.dram_tensor("out", [S, DQ], F32, kind="ExternalOutput").ap()

    with tile.TileContext(nc) as tc:
        with (
            tc.tile_pool(name="persist", bufs=1) as pp,
            tc.tile_pool(name="dram", bufs=1, space="DRAM") as dramp,
        ):
            # ---- persistent SBUF state
            qTr = [
                [
                    pp.tile([128, QW], BF16, tag=f"qTr{h}_{nq}", name=f"qTr{h}_{nq}")
                    for nq in range(NQUART)
                ]
                for h in range(HQ)
            ]
            kTr = [
                pp.tile([128, QW], BF16, tag=f"kTr{nq}", name=f"kTr{nq}")
                for nq in range(NQUART)
            ]
            # natural-layout V, 4 blocks per quarter, each [128, 129]
            # (last col = ones -> PV matmul also emits the softmax rowsum)
            vNat = [
                pp.tile([128, 4 * VB], BF16, tag=f"vNat{nq}", name=f"vNat{nq}")
                for nq in range(NQUART)
            ]
            tri_sb = pp.tile([128, 128], BF16, tag="tri", name="tri_sb")
            eye_sb = pp.tile([128, 128], BF16, tag="eye", name="eye_sb")
            wq_sb = pp.tile([128, KC * DQ], BF16, tag="wq", name="wq_sb")
            wk_sb = pp.tile([128, KC * HD], BF16, tag="wk", name="wk_sb")
            wv_sb = pp.tile([128, KC * HD], BF16, tag="wv", name="wv_sb")
            wo_sb = pp.tile([128, KC * DQ], BF16, tag="wo", name="wo_sb")

            # ---- DRAM collective buffers
            ag_ins = [
                dramp.tile([DQ, 256], BF16, tag=f"agin{c}", name=f"agin{c}")
                for c in range(NPAIR)
            ]
            ag_outs = [
                dramp.tile(
                    [H, 256], BF16, tag=f"agout{c}", name=f"agout{c}",
                    addr_space="Shared",
                )
                for c in range(NPAIR)
            ]

            # Warm up the CC stream immediately: the first collective pays a
            # ~100us all-core barrier + stream setup; two tiny AllGathers up
            # front let that overlap phase 1 instead of stalling o_proj.
            warm_in = dramp.tile([128, 8], BF16, tag="win", name="warm_in")
            warm_sb = pp.tile([128, 8], BF16, tag="wsb", name="warm_sb")
            nc.vector.memset(warm_sb[:], 0.0)
            nc.sync.dma_start(warm_in[:], warm_sb[:])
            warm_outs = [
                dramp.tile(
                    [N_CORES * 128, 8], BF16, tag=f"wout{w}", name=f"warm_out{w}",
                    addr_space="Shared",
                )
                for w in range(2)
            ]
            for w in range(2):
                nc.gpsimd.collective_compute(
                    "AllGather",
                    mybir.AluOpType.bypass,
                    replica_groups=[list(range(N_CORES))],
                    ins=[warm_in.opt()],
                    outs=[warm_outs[w].opt()],
                )

            nc.sync.dma_start(eye_sb[:], eye[:])
            nc.sync.dma_start(tri_sb[:], tri[:])
            # ones columns of vNat (written once, before any transposes land)
            for nq in range(NQUART):
                for b in range(4):
                    nc.vector.memset(vNat[nq][:, b * VB + 128 : b * VB + 129], 1.0)

            # Phase-2 SBUF pools first: they live below the phase-1 pools so
            # the phase boundary has no SBUF reuse hazards.
            with (
                tc.tile_pool(name="p2_e", bufs=3) as ep,
                tc.tile_pool(name="p2_sb", bufs=2) as asb,
                tc.tile_pool(name="p2_ag", bufs=16) as agp,
                tc.tile_pool(name="p2_ev", bufs=2) as evp,
            ):
                # ============= Phase 1: QKV projections + RoPE + V layout
                with (
                    tc.tile_pool(name="p1_stream", bufs=8) as stp,
                    tc.tile_pool(name="p1_small", bufs=2) as sp,
                    tc.tile_pool(name="p1_ps", bufs=1, space="PSUM") as pspp,
                    tc.tile_pool(name="tr_ps", bufs=2, space="PSUM") as trpp,
                ):
                    # qkv weights in c-chunk order interleaved with the
                    # quarter-0 hid stream; the first pieces are single
                    # chunks so the first matmul issues within ~10us
                    bounds = [0, 1, 2, 4, 6, 8] + list(range(12, KC + 1, 4))
                    pieces = list(zip(bounds[:-1], bounds[1:]))
                    hid_q0 = {}
                    for (a, b) in pieces:
                        nc.sync.dma_start(
                            wq_sb[:, a * DQ : b * DQ], wq_sw[:, a * DQ : b * DQ]
                        )
                        nc.sync.dma_start(
                            wk_sb[:, a * HD : b * HD], wk_sw[:, a * HD : b * HD]
                        )
                        nc.sync.dma_start(
                            wv_sb[:, a * HD : b * HD], wv_sw[:, a * HD : b * HD]
                        )
                        for c in range(a, min(b, NPRE)):
                            hc = stp.tile(
                                [128, QW], BF16, tag=f"hid0_{c}",
                                name=f"hid0_{c}", bufs=1,
                            )
                            nc.sync.dma_start(hc[:], hid_sw[:, c * S : c * S + QW])
                            hid_q0[c] = hc

                    for nq in range(NQUART):
                        ncols = slice(nq * QW, (nq + 1) * QW)
                        cos_sb = sp.tile([128, QW], F32, tag="cos", name="cos_sb")
                        sin_sb = sp.tile([128, QW], F32, tag="sin", name="sin_sb")
                        nc.sync.dma_start(cos_sb[:], cosF[:, ncols])
                        nc.sync.dma_start(sin_sb[:], sinS[:, ncols])

                        ps_q = [
                            pspp.tile([128, QW], F32, tag=f"psq{h}", name=f"psq{h}")
                            for h in range(HQ)
                        ]
                        ps_k = pspp.tile([128, QW], F32, tag="psk", name="ps_k")
                        ps_v = pspp.tile([128, QW], F32, tag="psv", name="ps_v")
                        for c in range(KC):
                            if nq == 0 and c < NPRE:
                                hid_c = hid_q0.pop(c)
                            else:
                                hid_c = stp.tile(
                                    [128, QW], BF16, tag="hid", name="hid_c"
                                )
                                nc.sync.dma_start(
                                    hid_c[:],
                                    hid_sw[:, c * S + nq * QW : c * S + (nq + 1) * QW],
                                )
                            st, sp_ = (c == 0), (c == KC - 1)
                            for h in range(HQ):
                                nc.tensor.matmul(
                                    ps_q[h][:],
                                    wq_sb[:, c * DQ + h * HD : c * DQ + (h + 1) * HD],
                                    hid_c[:],
                                    start=st,
                                    stop=sp_,
                                )
                            nc.tensor.matmul(
                                ps_k[:], wk_sb[:, c * HD : (c + 1) * HD], hid_c[:],
                                start=st, stop=sp_,
                            )
                            nc.tensor.matmul(
                                ps_v[:], wv_sb[:, c * HD : (c + 1) * HD], hid_c[:],
                                start=st, stop=sp_,
                            )

                        # V: evacuate to bf16, then 4 PE transposes into vNat
                        # (XBAR dma transpose corrupts unaligned dsts)
                        vT_q = sp.tile([128, QW], BF16, tag="vTq", name="vT_q")
                        nc.vector.tensor_copy(vT_q[:], ps_v[:])
                        for b in range(4):
                            tr = trpp.tile([128, 128], BF16, tag="tr", name="tr")
                            nc.tensor.transpose(
                                tr[:], vT_q[:, b * 128 : (b + 1) * 128], eye_sb[:]
                            )
                            nc.vector.tensor_copy(
                                vNat[nq][:, b * VB : b * VB + 128], tr[:]
                            )

                        # RoPE: dst = ps*cos + swap(ps)*sin. PSUM-reading ops
                        # (ACT copy + cos-mul) hoisted for all heads first so
                        # the PSUM banks free quickly. Temps in bf16.
                        srcs = [(ps_k, kTr[nq])] + [
                            (ps_q[h], qTr[h][nq]) for h in range(HQ)
                        ]
                        raws, t1s = [], []
                        for hi, (ps_x, _) in enumerate(srcs):
                            raw = sp.tile(
                                [128, QW], BF16, tag=f"raw{hi}", name=f"raw{hi}",
                                bufs=1,
                            )
                            t1 = sp.tile(
                                [128, QW], BF16, tag=f"t1_{hi}", name=f"t1_{hi}",
                                bufs=1,
                            )
                            nc.scalar.activation(
                                raw[:], ps_x[:], mybir.ActivationFunctionType.Copy
                            )
                            nc.vector.tensor_mul(t1[:], ps_x[:], cos_sb[:])
                            raws.append(raw)
                            t1s.append(t1)
                        for hi, (ps_x, dstT) in enumerate(srcs):
                            raw, t1 = raws[hi], t1s[hi]
                            swp = sp.tile(
                                [128, QW], BF16, tag=f"swp{hi}", name=f"swp{hi}",
                                bufs=1,
                            )
                            nc.sync.dma_start(swp[0:64, :], raw[64:128, :])
                            nc.sync.dma_start(swp[64:128, :], raw[0:64, :])
                            t2 = sp.tile([128, QW], BF16, tag="t2", name="t2")
                            nc.vector.tensor_mul(t2[:], swp[:], sin_sb[:])
                            nc.vector.tensor_add(dstT[:], t1[:], t2[:])

                        # trickle in wo while phase-1 compute runs
                        w = KC * DQ // NQUART
                        nc.sync.dma_start(
                            wo_sb[:, nq * w : (nq + 1) * w],
                            wo_sw[:, nq * w : (nq + 1) * w],
                        )

                # ============= Phase 2: attention + AllGather + o_proj,
                # software-pipelined: oproj(i-DEPTH) inside chunk i.
                with (
                    tc.tile_pool(name="s_ps", bufs=2, space="PSUM") as spsp,
                    tc.tile_pool(name="o_ps", bufs=2, space="PSUM") as opsp,
                    tc.tile_pool(name="op_ps", bufs=1, space="PSUM") as oppp,
                    tc.tile_pool(name="t_ps", bufs=2, space="PSUM") as trp2,
                ):
                    oproj_ps = {}
                    oproj_dmas = {}

                    def oproj_dma(i, sl):
                        """Prefetch the ag_sb tiles for o_proj slice sl."""
                        tiles = []
                        for c in range(8 * sl, 8 * sl + 8):
                            ag_sb = agp.tile(
                                [128, 256], BF16, tag="ag_sb", name="ag_sb"
                            )
                            nc.sync.dma_start(
                                ag_sb[:], ag_outs[i][c * 128 : (c + 1) * 128, :]
                            )
                            tiles.append(ag_sb)
                        oproj_dmas[(i, sl)] = tiles

                    def oproj_mm(i, sl):
                        """o_proj contraction chunks [8*sl, 8*sl+8) of chunk i."""
                        if sl == 0:
                            oproj_ps[i] = [
                                oppp.tile(
                                    [128, DQ], F32, tag=f"op{sb}", name=f"op{sb}"
                                )
                                for sb in range(2)
                            ]
                        ps01 = oproj_ps[i]
                        tiles = oproj_dmas.pop((i, sl))
                        for ci, c in enumerate(range(8 * sl, 8 * sl + 8)):
                            ag_sb = tiles[ci]
                            for sb in range(2):
                                nc.tensor.matmul(
                                    ps01[sb][:],
                                    ag_sb[:, sb * 128 : (sb + 1) * 128],
                                    wo_sb[:, c * DQ : (c + 1) * DQ],
                                    start=(c == 0),
                                    stop=(c == KC - 1),
                                )

                    def oproj_finish(i):
                        ps01 = oproj_ps.pop(i)
                        q0 = i * 256
                        for sb in range(2):
                            ev = evp.tile([128, DQ], F32, tag="ev", name="ev")
                            nc.vector.tensor_copy(ev[:], ps01[sb][:])
                            nc.sync.dma_start(
                                out[q0 + sb * 128 : q0 + (sb + 1) * 128, :], ev[:]
                            )

                    def attn_head(i, h):
                        q0 = i * 256
                        qq = q0 // QW          # quarter holding this pair
                        qbase = q0 - qq * QW
                        blocks = _pair_blocks(i)
                        widths = [
                            (128 if not (l and r) else 256) for (_, l, r) in blocks
                        ]
                        offs = list(np.cumsum([0] + widths))
                        e_t = ep.tile([128, 2304], BF16, tag="e", name="e_t")

                        # scores in 512-col PSUM groups -> exp -> e_t
                        g = 0
                        while g < len(blocks):
                            g_end = g
                            gw = 0
                            while g_end < len(blocks) and gw + widths[g_end] <= 512:
                                gw += widths[g_end]
                                g_end += 1
                            s_grp = spsp.tile([128, 512], F32, tag="sg", name="s_grp")
                            for bi in range(g, g_end):
                                j, l, r = blocks[bi]
                                qs = qbase if l else qbase + 128
                                w = widths[bi]
                                o = offs[bi] - offs[g]
                                nc.tensor.matmul(
                                    s_grp[:, o : o + w],
                                    kTr[j // 4][:, (j % 4) * 128 : (j % 4 + 1) * 128],
                                    qTr[h][qq][:, qs : qs + w],
                                    start=True,
                                    stop=True,
                                )
                            nc.scalar.activation(
                                e_t[:, offs[g] : offs[g] + gw],
                                s_grp[:, 0:gw],
                                mybir.ActivationFunctionType.Exp,
                                scale=SCALE,
                            )
                            g = g_end

                        # in-block causal masks on the two diagonal blocks
                        for bi, (j, l, r) in enumerate(blocks):
                            if j == 2 * i:
                                nc.vector.tensor_mul(
                                    e_t[:, offs[bi] : offs[bi] + 128],
                                    e_t[:, offs[bi] : offs[bi] + 128],
                                    tri_sb[:],
                                )
                            elif j == 2 * i + 1:
                                o = offs[bi] + (widths[bi] - 128)
                                nc.vector.tensor_mul(
                                    e_t[:, o : o + 128],
                                    e_t[:, o : o + 128],
                                    tri_sb[:],
                                )

                        # fused PV + rowsum: out_nat [q, 129] per subblock.
                        # One PSUM accumulation group may be pending per zero
                        # region: left group runs to completion, then right.
                        o_nat = opsp.tile([128, 2 * VB], F32, tag="on", name="o_nat")
                        nL = sum(1 for (_, l, _) in blocks if l)
                        nR = sum(1 for (_, _, r) in blocks if r)
                        cL = cR = 0
                        for bi, (j, l, r) in enumerate(blocks):
                            if not l:
                                continue
                            mv = vNat[j // 4][:, (j % 4) * VB : (j % 4) * VB + VB]
                            nc.tensor.matmul(
                                o_nat[:, 0:VB],
                                e_t[:, offs[bi] : offs[bi] + 128],
                                mv,
                                start=(cL == 0),
                                stop=(cL == nL - 1),
                            )
                            cL += 1
                        for bi, (j, l, r) in enumerate(blocks):
                            if not r:
                                continue
                            mv = vNat[j // 4][:, (j % 4) * VB : (j % 4) * VB + VB]
                            o = offs[bi] + (widths[bi] - 128)
                            nc.tensor.matmul(
                                o_nat[:, VB : 2 * VB],
                                e_t[:, o : o + 128],
                                mv,
                                start=(cR == 0),
                                stop=(cR == nR - 1),
                            )
                            cR += 1

                        # normalize per q row, transpose to [d, q] on the PE,
                        # ship to the AllGather input buffer
                        r_sb = asb.tile([128, 2], F32, tag="r", name="r_sb", bufs=4)
                        nc.vector.reciprocal(r_sb[:, 0:1], o_nat[:, 128:129])
                        nc.vector.reciprocal(
                            r_sb[:, 1:2], o_nat[:, 2 * VB - 1 : 2 * VB]
                        )
                        at_nat = asb.tile(
                            [128, 256], BF16, tag="an", name="at_nat", bufs=8
                        )
                        nc.vector.tensor_scalar_mul(
                            at_nat[:, 0:128], o_nat[:, 0:128], r_sb[:, 0:1]
                        )
                        nc.vector.tensor_scalar_mul(
                            at_nat[:, 128:256], o_nat[:, VB : VB + 128], r_sb[:, 1:2]
                        )
                        trT = trp2.tile([128, 256], BF16, tag="trT", name="trT")
                        nc.tensor.transpose(
                            trT[:, 0:128], at_nat[:, 0:128], eye_sb[:]
                        )
                        nc.tensor.transpose(
                            trT[:, 128:256], at_nat[:, 128:256], eye_sb[:]
                        )
                        at_cT = asb.tile(
                            [128, 256], BF16, tag="at", name="at_cT", bufs=8
                        )
                        nc.vector.tensor_copy(at_cT[:], trT[:])
                        nc.sync.dma_start(
                            ag_ins[i][h * 128 : (h + 1) * 128, :], at_cT[:]
                        )

                    for i in range(NPAIR):
                        for h in range(HQ):
                            if i >= DEPTH:
                                oproj_dma(i - DEPTH, h)
                            attn_head(i, h)
                            if i >= DEPTH:
                                oproj_mm(i - DEPTH, h)
                        nc.gpsimd.collective_compute(
                            "AllGather",
                            mybir.AluOpType.bypass,
                            replica_groups=[list(range(N_CORES))],
                            ins=[ag_ins[i].opt()],
                            outs=[ag_outs[i].opt()],
                        )
                        if i >= DEPTH:
                            oproj_finish(i - DEPTH)
                    for i in range(NPAIR - DEPTH, NPAIR):
                        oproj_dma(i, 0)
                        for sl in range(4):
                            if sl < 3:
                                oproj_dma(i, sl + 1)
                            oproj_mm(i, sl)
                        oproj_finish(i)

    nc.compile()
    return nc


@functools.lru_cache(maxsize=1)
def _cached_nc():
    return build_nc()


def _tables():
    pos = np.arange(S, dtype=np.float64)
    inv = 1.0 / (ROPE_BASE ** (np.arange(0, HD, 2, dtype=np.float64) / HD))  # [64]
    f = inv[:, None] * pos[None, :]                   # [64, S]
    cos = np.cos(f).astype(np.float32)
    sin = np.sin(f).astype(np.float32)
    cosF = np.concatenate([cos, cos], axis=0)         # [128, S]
    sinS = np.concatenate([-sin, sin], axis=0)        # [128, S]
    k_idx = np.arange(128)[:, None]
    q_idx = np.arange(128)[None, :]
    tri = (k_idx <= q_idx).astype(np.float32)         # [k, q] causal in-block
    return cosF, sinS, tri


def _swz(w: np.ndarray, bf16) -> np.ndarray:
    """[KC*128, W] -> chunk-major [128, KC*W] bf16."""
    kc, w_ = w.shape[0] // 128, w.shape[1]
    return np.ascontiguousarray(
        w.reshape(kc, 128, w_).transpose(1, 0, 2).reshape(128, kc * w_)
    ).astype(bf16)


def _run(hidden_states, wq, wk, wv, wo, **run_kwargs):
    nc = _cached_nc()
    bf16 = mybir.dt.np(BF16)
    # hid_sw[p, c*S + s] = hidden[s, c*128 + p]
    hid2 = np.asarray(hidden_states, dtype=np.float32).reshape(S, H)
    hid_sw = np.ascontiguousarray(
        hid2.reshape(S, KC, 128).transpose(2, 1, 0).reshape(128, KC * S)
    ).astype(bf16)
    cosF, sinS, tri = _tables()
    in_maps = []
    for c in range(N_CORES):
        in_maps.append(
            {
                "hid_sw": hid_sw,
                "wq_sw": _swz(wq[:, c * DQ : (c + 1) * DQ], bf16),
                "wk_sw": _swz(wk[:, c * HD : (c + 1) * HD], bf16),
                "wv_sw": _swz(wv[:, c * HD : (c + 1) * HD], bf16),
                "wo_sw": _swz(wo[:, c * DQ : (c + 1) * DQ], bf16),
                "cosF": cosF,
                "sinS": sinS,
                "tri": tri.astype(bf16),
                "eye": np.eye(128, dtype=np.float32).astype(bf16),
            }
        )
    res = run_bass_kernel_spmd(
        nc, in_maps, core_ids=list(range(N_CORES)), **run_kwargs
    )
    full = np.concatenate(
        [res.results[r]["out"] for r in range(N_CORES)], axis=1
    )
    return full.reshape(B, S, H).astype(np.float32), res


def kernel(hidden_states, wq, wk, wv, wo):
    out, _ = _run(hidden_states, wq, wk, wv, wo)
    return out



# revision 33
# speedup vs baseline: 1.0141x; 1.0141x over previous
"""Trainium2 Bass kernel for nn_LlamaAttention_48816598286577.

Llama attention with block-streaming sparse mask (sink=1 block, local
window=8 blocks, BLOCK=128), B=1 S=2048 H=4096, 32 q heads / 8 kv heads,
head_dim 128, non-interleaved RoPE.

Sharding: tensor-parallel over heads across 8 cores (4 q heads + 1 kv
head per core). All compute in bf16 (PSUM accumulates f32).

v2 structure (vs the phase-separated baseline): the sequence is processed
in 8 windows of 256 positions (= one query-block pair each). Window w
computes the QKV projections + RoPE for its 256 positions in two passes
(pass A: q0,q1,k; pass B: q2,q3,v) so projections only hold 3 PSUM banks.
Attention for pair w-1 is interleaved, a few matmuls at a time, into
window w's projection stream, and its AllGather is issued at the end of
window w -- ~150us earlier than the old end-of-phase-1 schedule, so the
CC stream (which costs ~20us per AllGather and ~88us for the first op
after idle) runs concurrently with the projections instead of serializing
the o_proj endgame.  o_proj runs as a tail, consuming the gathered
attention outputs through XBAR transpose DMAs (so no PE transposes of the
attention outputs are needed; V transposes stay on the PE).

PSUM budget (8 banks): 3 proj + 2 score-groups (shared with V transpose)
+ 1 PV accumulator + 2 o_proj accumulators.
"""

import functools
from collections import deque

import numpy as np

import concourse.bass as bass
import concourse.mybir as mybir
import concourse.tile as tile
from concourse import bacc
from concourse.bass_utils import run_bass_kernel_spmd

# problem constants (hardcoded per contract)
B, S, H = 1, 2048, 4096
NQ, NKV, HD = 32, 8, 128
BLOCK = 128
NBLK = S // BLOCK          # 16
SINK_BLOCKS = 1
LOCAL_BLOCKS = 8
ROPE_BASE = 10000.0
N_CORES = 8
HQ = NQ // N_CORES         # 4 q heads per core
DQ = HQ * HD               # 512 q columns per core
SCALE = 1.0 / float(np.sqrt(HD))

KC = H // 128              # 32 contraction chunks for projections
NPAIR = NBLK // 2          # 8 query pairs of 256
WW = 256                   # window width = one pair of q blocks

F32 = mybir.dt.float32
BF16 = mybir.dt.bfloat16

VB = 128                   # v-block stride in vNat


def _pair_blocks(i: int):
    """Key blocks for query pair i with per-block subblock coverage."""
    out = []
    for j in range(2 * i + 2):
        left = j <= 2 * i and (2 * i - j < LOCAL_BLOCKS or j < SINK_BLOCKS)
        right = j <= 2 * i + 1 and (2 * i + 1 - j < LOCAL_BLOCKS or j < SINK_BLOCKS)
        if left or right:
            out.append((j, left, right))
    return out


def _groups(blocks):
    """Score groups of 2 uniform 256-wide blocks (one PSUM bank)."""
    return [(g, min(g + 2, len(blocks))) for g in range(0, len(blocks), 2)]


class _IL:
    """Round-robin generator interleaver: pump() emits one quantum."""

    def __init__(self):
        self.q = deque()

    def add(self, gen):
        self.q.append(gen)

    def pump(self):
        while self.q:
            try:
                next(self.q[0])
                return True
            except StopIteration:
                self.q.popleft()
        return False

    def drain(self):
        while self.pump():
            pass


def build_nc():
    nc = bacc.Bacc(
        "TRN2", target_bir_lowering=False, debug=False, num_devices=N_CORES
    )
    hid_sw = nc.dram_tensor("hid_sw", [128, KC * S], BF16, kind="ExternalInput").ap()
    wq_sw = nc.dram_tensor("wq_sw", [128, KC * DQ], BF16, kind="ExternalInput").ap()
    wk_sw = nc.dram_tensor("wk_sw", [128, KC * HD], BF16, kind="ExternalInput").ap()
    wv_sw = nc.dram_tensor("wv_sw", [128, KC * HD], BF16, kind="ExternalInput").ap()
    wo_sw = nc.dram_tensor("wo_sw", [128, KC * DQ], BF16, kind="ExternalInput").ap()
    cosF = nc.dram_tensor("cosF", [128, S], F32, kind="ExternalInput").ap()
    sinS = nc.dram_tensor("sinS", [128, S], F32, kind="ExternalInput").ap()
    tri = nc.dram_tensor("tri", [128, 128], BF16, kind="ExternalInput").ap()
    eye = nc.dram_tensor("eye", [128, 128], BF16, kind="ExternalInput").ap()
    out = nc.dram_tensor("out", [S, DQ], F32, kind="ExternalOutput").ap()

    hid_r = hid_sw.rearrange("p (c s) -> p c s", c=KC)

    with tile.TileContext(nc) as tc:
        with (
            tc.tile_pool(name="persist", bufs=1) as pp,
            tc.tile_pool(name="dram", bufs=1, space="DRAM") as dramp,
        ):
            # ---- persistent SBUF state
            qTr = [
                [
                    pp.tile([128, 2 * WW], BF16, tag=f"qTr{h}_{nq}", name=f"qTr{h}_{nq}")
                    for nq in range(4)
                ]
                for h in range(HQ)
            ]
            kTr = [
                pp.tile([128, 2 * WW], BF16, tag=f"kTr{nq}", name=f"kTr{nq}")
                for nq in range(4)
            ]
            vNat = [
                pp.tile([128, 4 * VB], BF16, tag=f"vNat{nq}", name=f"vNat{nq}")
                for nq in range(4)
            ]
            tri_sb = pp.tile([128, 128], BF16, tag="tri", name="tri_sb")
            eye_sb = pp.tile([128, 128], BF16, tag="eye", name="eye_sb")
            wq_sb = pp.tile([128, KC * DQ], BF16, tag="wq", name="wq_sb")
            wk_sb = pp.tile([128, KC * HD], BF16, tag="wk", name="wk_sb")
            wv_sb = pp.tile([128, KC * HD], BF16, tag="wv", name="wv_sb")
            wo_sb = pp.tile([128, KC * DQ], BF16, tag="wo", name="wo_sb")
            cos_all = pp.tile([128, S], F32, tag="cos", name="cos_all")
            sin_all = pp.tile([128, S], F32, tag="sin", name="sin_all")

            # ---- DRAM collective buffers: [hd, q] layout (at transposed
            # on the PE before the AllGather)
            ag_ins = [
                dramp.tile([DQ, 256], BF16, tag=f"agin{c}", name=f"agin{c}")
                for c in range(NPAIR)
            ]
            ag_outs = [
                dramp.tile(
                    [H, 256], BF16, tag=f"agout{c}", name=f"agout{c}",
                    addr_space="Shared",
                )
                for c in range(NPAIR)
            ]

            # Warm up the CC stream immediately (first collective pays a
            # ~88us barrier; keep the stream busy until the first real AG).
            warm_in = dramp.tile([128, 8], BF16, tag="win", name="warm_in")
            warm_sb = pp.tile([128, 8], BF16, tag="wsb", name="warm_sb")
            nc.vector.memset(warm_sb[:], 0.0)
            nc.sync.dma_start(warm_in[:], warm_sb[:])
            warm_outs = [
                dramp.tile(
                    [N_CORES * 128, 8], BF16, tag=f"wout{w}", name=f"warm_out{w}",
                    addr_space="Shared",
                )
                for w in range(2)
            ]
            for w in range(2):
                nc.gpsimd.collective_compute(
                    "AllGather",
                    mybir.AluOpType.bypass,
                    replica_groups=[list(range(N_CORES))],
                    ins=[warm_in.opt()],
                    outs=[warm_outs[w].opt()],
                )

            nc.sync.dma_start(eye_sb[:], eye[:])
            nc.sync.dma_start(tri_sb[:], tri[:])
            ones_sb = pp.tile([128, 128], BF16, tag="ones", name="ones_sb")
            nc.vector.memset(ones_sb[:], 1.0)

            with (
                tc.tile_pool(name="hidp", bufs=2) as hidp,
                tc.tile_pool(name="small", bufs=2) as sp,
                tc.tile_pool(name="ep", bufs=3) as ep,
                tc.tile_pool(name="asb", bufs=4) as asb,
                tc.tile_pool(name="agp", bufs=4) as agp,
                tc.tile_pool(name="evp", bufs=2) as evp,
                tc.tile_pool(name="pjp", bufs=1, space="PSUM") as pjp,
                tc.tile_pool(name="sgp", bufs=2, space="PSUM") as sgp,
                tc.tile_pool(name="onp", bufs=1, space="PSUM") as onp,
                tc.tile_pool(name="opp", bufs=1, space="PSUM") as opp,
            ):
                # ---------- staging: weights + window-0 hid interleaved
                hid_tiles = {}
                h0 = hidp.tile([128, KC * WW], BF16, tag="hid", name="hid_w0")
                hid_tiles[0] = h0
                h0_r = h0.rearrange("p (c s) -> p c s", c=KC)
                bounds = [0, 1, 2, 4, 6, 8] + list(range(12, KC + 1, 4))
                pieces = list(zip(bounds[:-1], bounds[1:]))
                def stage_piece(a, b):
                    nc.sync.dma_start(
                        wq_sb[:, a * DQ : b * DQ], wq_sw[:, a * DQ : b * DQ]
                    )
                    nc.sync.dma_start(
                        wk_sb[:, a * HD : b * HD], wk_sw[:, a * HD : b * HD]
                    )
                    nc.sync.dma_start(
                        wv_sb[:, a * HD : b * HD], wv_sw[:, a * HD : b * HD]
                    )
                    nc.sync.dma_start(h0_r[:, a:b, :], hid_r[:, a:b, 0:WW])

                for (a, b) in pieces:
                    if a >= 16:
                        break
                    stage_piece(a, b)
                # window-1 hid early, ahead of the second half of the
                # weights, so window 1 never waits on it
                h1 = hidp.tile([128, KC * WW], BF16, tag="hid", name="hid_w1")
                h1_r = h1.rearrange("p (c s) -> p c s", c=KC)
                nc.sync.dma_start(h1_r[:], hid_r[:, :, WW : 2 * WW])
                hid_tiles[1] = h1
                for (a, b) in pieces:
                    if a >= 16:
                        stage_piece(a, b)
                nc.sync.dma_start(cos_all[:, 0 : S // 2], cosF[:, 0 : S // 2])
                nc.sync.dma_start(sin_all[:, 0 : S // 2], sinS[:, 0 : S // 2])

                il = _IL()
                ag_issued = [False] * NPAIR

                def issue_ag(p):
                    nc.gpsimd.collective_compute(
                        "AllGather",
                        mybir.AluOpType.bypass,
                        replica_groups=[list(range(N_CORES))],
                        ins=[ag_ins[p].opt()],
                        outs=[ag_outs[p].opt()],
                    )
                    ag_issued[p] = True

                def ag_gen(p):
                    # interleaver sentinel: issues AG[p] when the FIFO
                    # reaches it (i.e. right after pair p's units finish)
                    issue_ag(p)
                    return
                    yield

                def attn_unit(p, h):
                    """Generator: attention for pair p, head h, in quanta.

                    All key blocks are computed 256 wide (both q subblocks);
                    invalid halves are zeroed after exp.  PV runs transposed
                    (lhsT = V block) so the output lands directly in the
                    [d, q] AllGather layout -- no PE output transposes.  The
                    softmax denominators come from 1-column-stationary
                    matmuls accumulated in the right half of the PV bank
                    (start=True of the first PV matmul clears the whole
                    bank's has_written bits, so the rowsum chain uses
                    start=False throughout).
                    """
                    q0 = p * WW
                    qq = q0 // 512
                    qbase = q0 - qq * 512
                    blocks = _pair_blocks(p)
                    nb = len(blocks)
                    e_t = ep.tile([128, 2560], BF16, tag="e", name="e_t")

                    for (g, g_end) in _groups(blocks):
                        s_grp = sgp.tile([128, 512], F32, tag="sg", name="s_grp")
                        for bi in range(g, g_end):
                            j, l, r = blocks[bi]
                            # half-valid blocks only compute the valid half;
                            # the stale other half is zeroed after exp
                            qo = 0 if l else 128
                            qw = 256 if (l and r) else 128
                            nc.tensor.matmul(
                                s_grp[
                                    :,
                                    (bi - g) * 256 + qo : (bi - g) * 256 + qo + qw,
                                ],
                                kTr[j // 4][:, (j % 4) * 128 : (j % 4 + 1) * 128],
                                qTr[h][qq][:, qbase + qo : qbase + qo + qw],
                                start=True,
                                stop=True,
                            )
                        gw = (g_end - g) * 256
                        nc.scalar.activation(
                            e_t[:, g * 256 : g * 256 + gw],
                            s_grp[:, 0:gw],
                            mybir.ActivationFunctionType.Exp,
                            scale=SCALE,
                        )
                        for bi in range(g, g_end):
                            j, l, r = blocks[bi]
                            o = bi * 256
                            if not l:
                                nc.vector.memset(e_t[:, o : o + 128], 0.0)
                            elif j == 2 * p:
                                nc.vector.tensor_mul(
                                    e_t[:, o : o + 128],
                                    e_t[:, o : o + 128],
                                    tri_sb[:],
                                )
                            if not r:
                                nc.vector.memset(e_t[:, o + 128 : o + 256], 0.0)
                            elif j == 2 * p + 1:
                                nc.vector.tensor_mul(
                                    e_t[:, o + 128 : o + 256],
                                    e_t[:, o + 128 : o + 256],
                                    tri_sb[:],
                                )
                        yield

                    # PV (transposed): o_both[:, 0:256] = sum_b V_b.T @ e_b
                    # ([d, q]).  Denominators: DVE folds the blocks
                    # (e_sum[k,q] = sum_b e_b[k,q]), then ONE all-ones matmul
                    # does the k-reduction into o_both[:, 256:512] (broadcast
                    # across partitions by the all-ones lhsT) -- saving ~9
                    # rowsum matmuls per unit on the throttled PE.
                    o_both = onp.tile([128, 512], F32, tag="on", name="o_both")
                    half = nb // 2
                    es_parts = []
                    for s, e in ((0, half), (half, nb)):
                        for bi in range(s, e):
                            j, _, _ = blocks[bi]
                            nc.tensor.matmul(
                                o_both[:, 0:256],
                                vNat[j // 4][:, (j % 4) * VB : (j % 4) * VB + 128],
                                e_t[:, bi * 256 : (bi + 1) * 256],
                                start=(bi == 0),
                                stop=(bi == nb - 1),
                            )
                        # partial denominator folds (split so no single DVE
                        # op exceeds ~1.7us and blocks RoPE ops in the FIFO)
                        es = asb.tile(
                            [128, 256], F32, tag=f"es{s != 0}",
                            name="es", bufs=1,
                        )
                        if e - s > 1:
                            nc.vector.tensor_reduce(
                                es[:],
                                e_t[:, s * 256 : e * 256].rearrange(
                                    "p (b q) -> p q b", b=e - s
                                ),
                                axis=mybir.AxisListType.X,
                                op=mybir.AluOpType.add,
                            )
                        else:
                            nc.vector.tensor_copy(es[:], e_t[:, s * 256 : e * 256])
                        es_parts.append(es)
                        yield

                    e_sum = asb.tile(
                        [128, 256], BF16, tag="esb", name="e_sum", bufs=2
                    )
                    nc.vector.tensor_add(e_sum[:], es_parts[0][:], es_parts[1][:])
                    nc.tensor.matmul(
                        o_both[:, 256:512],
                        ones_sb[:],
                        e_sum[:],
                        start=False,
                        stop=True,
                        skip_group_check=True,
                    )
                    rbr = asb.tile([128, 256], F32, tag="rbr", name="rbr", bufs=2)
                    nc.vector.reciprocal_approx_fast(rbr[:], o_both[:, 256:512])
                    at_cT = asb.tile(
                        [128, 256], BF16, tag="at", name="at_cT", bufs=8
                    )
                    nc.vector.tensor_mul(at_cT[:], o_both[:, 0:256], rbr[:])
                    nc.sync.dma_start(
                        ag_ins[p][h * 128 : (h + 1) * 128, :], at_cT[:]
                    )
                    yield

                def unit_quanta(p):
                    return len(_groups(_pair_blocks(p))) + 4

                # ---------- RoPE per window pass
                def rope_pass(w, srcs):
                    """srcs: list of (idx, psum_tile, dstT_slice)."""
                    cw = slice(w * WW, (w + 1) * WW)
                    raws = []
                    for idx, ps_x, dstT in srcs:
                        raw = sp.tile(
                            [128, WW], BF16, tag=f"raw{idx}", name=f"raw{idx}"
                        )
                        nc.vector.tensor_copy(raw[:], ps_x[:])  # sole PSUM read
                        raws.append(raw)
                    for (idx, ps_x, dstT), raw in zip(srcs, raws):
                        swp = sp.tile(
                            [128, WW], BF16, tag=f"swp{idx}", name=f"swp{idx}"
                        )
                        nc.sync.dma_start(swp[0:64, :], raw[64:128, :])
                        nc.sync.dma_start(swp[64:128, :], raw[0:64, :])
                        t1 = sp.tile([128, WW], BF16, tag=f"t1_{idx}", name=f"t1_{idx}")
                        nc.vector.tensor_mul(t1[:], raw[:], cos_all[:, cw])
                        t2 = sp.tile([128, WW], BF16, tag="t2", name="t2", bufs=4)
                        nc.vector.tensor_mul(t2[:], swp[:], sin_all[:, cw])
                        nc.vector.tensor_add(dstT[:], t1[:], t2[:])

                # ---------- window loop
                deferred_pe = []  # V transposes deferred into next window

                for w in range(NPAIR):
                    if 1 <= w and w + 1 < NPAIR:
                        h2 = hidp.tile(
                            [128, KC * WW], BF16, tag="hid", name=f"hid_w{w+1}"
                        )
                        h2_r = h2.rearrange("p (c s) -> p c s", c=KC)
                        nc.sync.dma_start(
                            h2_r[:], hid_r[:, :, (w + 1) * WW : (w + 2) * WW]
                        )
                        hid_tiles[w + 1] = h2
                    if w == 2:
                        nc.sync.dma_start(
                            cos_all[:, S // 2 :], cosF[:, S // 2 :]
                        )
                        nc.sync.dma_start(
                            sin_all[:, S // 2 :], sinS[:, S // 2 :]
                        )
                    if 3 <= w < 7:
                        # trickle wo in quarters during windows 3-6 (needed
                        # only by the o_proj tail; keeps it off the
                        # congested startup HBM window)
                        q = KC * DQ // 4
                        nc.sync.dma_start(
                            wo_sb[:, (w - 3) * q : (w - 2) * q],
                            wo_sw[:, (w - 3) * q : (w - 2) * q],
                        )

                    hid_c = hid_tiles.pop(w)
                    npend = 4 * unit_quanta(w - 1) if w >= 1 else 0
                    # pump positions among the 64 chunk-passes (start late
                    # enough that RoPE of window w-1 has landed)
                    positions = {}
                    if npend:
                        span = 64 - 6
                        for k in range(npend):
                            pos = 5 + (k * span) // npend
                            positions[pos] = positions.get(pos, 0) + 1

                    qq, half = w // 2, w % 2
                    cp = 0
                    for pas in range(2):
                        ha, hb = (0, 1) if pas == 0 else (2, 3)
                        ps_a = pjp.tile([128, WW], F32, tag="pa0", name="ps_a")
                        ps_b = pjp.tile([128, WW], F32, tag="pa1", name="ps_b")
                        ps_kv = pjp.tile([128, WW], F32, tag="pk", name="ps_kv")
                        wkv_sb = wk_sb if pas == 0 else wv_sb
                        for c in range(KC):
                            st, sp_ = (c == 0), (c == KC - 1)
                            hs = hid_c[:, c * WW : (c + 1) * WW]
                            nc.tensor.matmul(
                                ps_a[:],
                                wq_sb[:, c * DQ + ha * HD : c * DQ + (ha + 1) * HD],
                                hs, start=st, stop=sp_,
                            )
                            nc.tensor.matmul(
                                ps_b[:],
                                wq_sb[:, c * DQ + hb * HD : c * DQ + (hb + 1) * HD],
                                hs, start=st, stop=sp_,
                            )
                            nc.tensor.matmul(
                                ps_kv[:], wkv_sb[:, c * HD : (c + 1) * HD], hs,
                                start=st, stop=sp_,
                            )
                            if cp == 2 and deferred_pe:
                                for fn in deferred_pe:
                                    fn()
                                deferred_pe = []
                            for _ in range(positions.get(cp, 0)):
                                il.pump()
                            cp += 1

                        dsl = slice(half * WW, (half + 1) * WW)
                        if pas == 0:
                            srcs = [
                                (2, ps_kv, kTr[qq][:, dsl]),
                                (0, ps_a, qTr[0][qq][:, dsl]),
                                (1, ps_b, qTr[1][qq][:, dsl]),
                            ]
                            rope_pass(w, srcs)
                        else:
                            srcs = [
                                (3, ps_a, qTr[2][qq][:, dsl]),
                                (4, ps_b, qTr[3][qq][:, dsl]),
                            ]
                            rope_pass(w, srcs)
                            # V: evacuate + 2 PE transposes (deferred into
                            # the next window so the PE never waits here)
                            vT_q = sp.tile([128, WW], BF16, tag="vT", name="vT_q")
                            nc.vector.tensor_copy(vT_q[:], ps_kv[:])

                            def v_tr(w=w, vT_q=vT_q, qq=qq, half=half):
                                for b_ in range(2):
                                    tr = sgp.tile(
                                        [128, 128], BF16, tag="sg", name="tr"
                                    )
                                    nc.tensor.transpose(
                                        tr[:],
                                        vT_q[:, b_ * 128 : (b_ + 1) * 128],
                                        eye_sb[:],
                                    )
                                    blk = 2 * half + b_
                                    nc.vector.tensor_copy(
                                        vNat[qq][:, blk * VB : blk * VB + 128], tr[:]
                                    )

                            deferred_pe.append(v_tr)

                    il.drain()
                    if w + 1 < NPAIR:
                        for h in range(HQ):
                            il.add(attn_unit(w, h))
                        il.add(ag_gen(w))

                # ---------- tail: pair-7 attention + o_proj
                for fn in deferred_pe:
                    fn()
                deferred_pe = []
                for h in range(HQ):
                    il.add(attn_unit(NPAIR - 1, h))
                il.add(ag_gen(NPAIR - 1))

                def ensure_ag(p):
                    while not ag_issued[p]:
                        if not il.pump():
                            raise RuntimeError(f"AG {p} never issued")

                ag_sbs = {}

                def oproj_dma(p, r):
                    # One batched load per slice on the GpSimd (SWDGE)
                    # queue: nothing latency-critical shares that queue, so
                    # its waits on AllGather completion can't block
                    # exp/RoPE/hid DMAs, and batching keeps the Q7 issue
                    # cost (~0.6us per dma_start) off the tail critical
                    # path.
                    t = agp.tile([128, 4 * 256], BF16, tag="ag", name="ag_sb", bufs=4)
                    t_r = t.rearrange("p (c q) -> p c q", c=4)
                    src = ag_outs[p][512 * r : 512 * (r + 1), :].rearrange(
                        "(c p) q -> p c q", c=4
                    )
                    nc.gpsimd.dma_start(t_r, src)
                    ag_sbs[(p, r)] = t

                def oproj_mm(p, r, ps01):
                    # 4 N=256 accumulation chains (2 col-halves per bank):
                    # N=256 LDW+MM chains sustain ~110-140ns/MM on this HW
                    # where N=512 pairs measure ~283ns.
                    t = ag_sbs.pop((p, r))
                    for c2 in range(4):
                        c = 4 * r + c2
                        for sb in range(2):
                            for ch in range(2):
                                nc.tensor.matmul(
                                    ps01[sb][:, ch * 256 : (ch + 1) * 256],
                                    t[:, c2 * 256 + sb * 128 : c2 * 256 + (sb + 1) * 128],
                                    wo_sb[:, c * DQ + ch * 256 : c * DQ + (ch + 1) * 256],
                                    start=(c == 0 and ch == 0),
                                    stop=(c == KC - 1),
                                    skip_group_check=(ch == 1),
                                )

                def oproj_finish(p, ps01):
                    q0 = p * 256
                    for sb in range(2):
                        ev = evp.tile([128, DQ], F32, tag="ev", name="ev")
                        nc.vector.tensor_copy(ev[:], ps01[sb][:])
                        nc.sync.dma_start(
                            out[q0 + sb * 128 : q0 + (sb + 1) * 128, :], ev[:]
                        )

                seq = [(p, r) for p in range(NPAIR) for r in range(N_CORES)]
                ensure_ag(seq[0][0])
                oproj_dma(*seq[0])
                oproj_dma(*seq[1])
                ps_map = {}
                for k, (p, r) in enumerate(seq):
                    if k + 2 < len(seq):
                        ensure_ag(seq[k + 2][0])
                        oproj_dma(*seq[k + 2])
                    if r == 0:
                        ps_map[p] = [
                            opp.tile([128, DQ], F32, tag=f"op{sb}", name=f"op{sb}")
                            for sb in range(2)
                        ]
                    oproj_mm(p, r, ps_map[p])
                    il.pump()
                    il.pump()
                    if r == N_CORES - 1:
                        oproj_finish(p, ps_map.pop(p))
                il.drain()
                assert all(ag_issued)

    nc.compile()
    return nc


@functools.lru_cache(maxsize=1)
def _cached_nc():
    return build_nc()


def _tables():
    pos = np.arange(S, dtype=np.float64)
    inv = 1.0 / (ROPE_BASE ** (np.arange(0, HD, 2, dtype=np.float64) / HD))  # [64]
    f = inv[:, None] * pos[None, :]                   # [64, S]
    cos = np.cos(f).astype(np.float32)
    sin = np.sin(f).astype(np.float32)
    cosF = np.concatenate([cos, cos], axis=0)         # [128, S]
    sinS = np.concatenate([-sin, sin], axis=0)        # [128, S]
    k_idx = np.arange(128)[:, None]
    q_idx = np.arange(128)[None, :]
    tri = (k_idx <= q_idx).astype(np.float32)         # [k, q] causal in-block
    return cosF, sinS, tri


def _swz(w: np.ndarray, bf16) -> np.ndarray:
    """[KC*128, W] -> chunk-major [128, KC*W] bf16."""
    kc, w_ = w.shape[0] // 128, w.shape[1]
    return np.ascontiguousarray(
        w.reshape(kc, 128, w_).transpose(1, 0, 2).reshape(128, kc * w_)
    ).astype(bf16)


def _run(hidden_states, wq, wk, wv, wo, **run_kwargs):
    nc = _cached_nc()
    bf16 = mybir.dt.np(BF16)
    # hid_sw[p, c*S + s] = hidden[s, c*128 + p]
    hid2 = np.asarray(hidden_states, dtype=np.float32).reshape(S, H)
    hid_sw = np.ascontiguousarray(
        hid2.reshape(S, KC, 128).transpose(2, 1, 0).reshape(128, KC * S)
    ).astype(bf16)
    cosF, sinS, tri = _tables()
    in_maps = []
    for c in range(N_CORES):
        in_maps.append(
            {
                "hid_sw": hid_sw,
                "wq_sw": _swz(wq[:, c * DQ : (c + 1) * DQ], bf16),
                "wk_sw": _swz(wk[:, c * HD : (c + 1) * HD], bf16),
                "wv_sw": _swz(wv[:, c * HD : (c + 1) * HD], bf16),
                "wo_sw": _swz(wo[:, c * DQ : (c + 1) * DQ], bf16),
                "cosF": cosF,
                "sinS": sinS,
                "tri": tri.astype(bf16),
                "eye": np.eye(128, dtype=np.float32).astype(bf16),
            }
        )
    res = run_bass_kernel_spmd(
        nc, in_maps, core_ids=list(range(N_CORES)), **run_kwargs
    )
    full = np.concatenate(
        [res.results[r]["out"] for r in range(N_CORES)], axis=1
    )
    return full.reshape(B, S, H).astype(np.float32), res


def kernel(hidden_states, wq, wk, wv, wo):
    out, _ = _run(hidden_states, wq, wk, wv, wo)
    return out


# revision 34
# speedup vs baseline: 1.0369x; 1.0225x over previous
"""Trainium2 Bass kernel for nn_LlamaAttention_48816598286577.

Llama attention with block-streaming sparse mask (sink=1 block, local
window=8 blocks, BLOCK=128), B=1 S=2048 H=4096, 32 q heads / 8 kv heads,
head_dim 128, non-interleaved RoPE.

Sharding: tensor-parallel over heads across 8 cores (4 q heads + 1 kv
head per core). All compute in bf16 (PSUM accumulates f32).

Structure (vs the phase-separated baseline): the sequence is processed
in 8 windows of 256 positions (= one query-block pair each). Window w
computes the QKV projections + RoPE for its 256 positions in two passes
(pass A: q0,q1,k; pass B: q2,q3,v) so projections only hold 3 PSUM banks.
Attention for pair w-1 is interleaved, a few matmuls at a time, into
window w's projection stream (a generator-based round-robin), and its
AllGather is issued mid-window -- ~150us earlier than an end-of-phase-1
schedule, so the CC stream (~20-30us per AllGather, ~88us for the first
op after idle) runs concurrently with the projections instead of
serializing the o_proj endgame.

Attention PV runs TRANSPOSED (lhsT = V block), so the output lands
directly in the [d, q] layout the AllGather wants -- no PE output
transposes.  Key blocks are uniform 256 wide; invalid halves are zeroed
after exp.  Softmax denominators: DVE folds the blocks of exp scores,
then one all-ones matmul does the k-reduction, its output broadcast
across all partitions by construction; normalization is a fast-approx
reciprocal + one multiply.  o_proj runs as a tail of N=256 accumulation
chains, with its gather loads batched on the otherwise-idle GpSimd
(SWDGE) queue so AllGather-completion waits never block the Sync/Scalar
queues (exp, RoPE-swap and hid DMAs live there).

PSUM budget (8 banks): 3 proj + 2 score-groups (shared with V transpose)
+ 1 PV/denominator bank + 2 o_proj accumulators.

Measured (same-session, NTFF-profiled): ~502-515us vs 522-555us for the
phase-separated baseline.  The PE runs power-throttled at k=13/16
(~1.95 GHz) for most of the kernel; at ~140ns per 256-column matmul the
kernel sits within ~10% of that throttled issue-rate floor.
"""

import functools
from collections import deque

import numpy as np

import concourse.bass as bass
import concourse.mybir as mybir
import concourse.tile as tile
from concourse import bacc
from concourse.bass_utils import run_bass_kernel_spmd

# problem constants (hardcoded per contract)
B, S, H = 1, 2048, 4096
NQ, NKV, HD = 32, 8, 128
BLOCK = 128
NBLK = S // BLOCK          # 16
SINK_BLOCKS = 1
LOCAL_BLOCKS = 8
ROPE_BASE = 10000.0
N_CORES = 8
HQ = NQ // N_CORES         # 4 q heads per core
DQ = HQ * HD               # 512 q columns per core
SCALE = 1.0 / float(np.sqrt(HD))

KC = H // 128              # 32 contraction chunks for projections
NPAIR = NBLK // 2          # 8 query pairs of 256
WW = 256                   # window width = one pair of q blocks

F32 = mybir.dt.float32
BF16 = mybir.dt.bfloat16

VB = 128                   # v-block stride in vNat


def _pair_blocks(i: int):
    """Key blocks for query pair i with per-block subblock coverage."""
    out = []
    for j in range(2 * i + 2):
        left = j <= 2 * i and (2 * i - j < LOCAL_BLOCKS or j < SINK_BLOCKS)
        right = j <= 2 * i + 1 and (2 * i + 1 - j < LOCAL_BLOCKS or j < SINK_BLOCKS)
        if left or right:
            out.append((j, left, right))
    return out


def _groups(blocks):
    """Score groups of 2 uniform 256-wide blocks (one PSUM bank)."""
    return [(g, min(g + 2, len(blocks))) for g in range(0, len(blocks), 2)]


class _IL:
    """Round-robin generator interleaver: pump() emits one quantum."""

    def __init__(self):
        self.q = deque()

    def add(self, gen):
        self.q.append(gen)

    def pump(self):
        while self.q:
            try:
                next(self.q[0])
                return True
            except StopIteration:
                self.q.popleft()
        return False

    def drain(self):
        while self.pump():
            pass


def build_nc():
    nc = bacc.Bacc(
        "TRN2", target_bir_lowering=False, debug=False, num_devices=N_CORES
    )
    hid_sw = nc.dram_tensor("hid_sw", [128, KC * S], BF16, kind="ExternalInput").ap()
    wq_sw = nc.dram_tensor("wq_sw", [128, KC * DQ], BF16, kind="ExternalInput").ap()
    wk_sw = nc.dram_tensor("wk_sw", [128, KC * HD], BF16, kind="ExternalInput").ap()
    wv_sw = nc.dram_tensor("wv_sw", [128, KC * HD], BF16, kind="ExternalInput").ap()
    wo_sw = nc.dram_tensor("wo_sw", [128, KC * DQ], BF16, kind="ExternalInput").ap()
    cosF = nc.dram_tensor("cosF", [128, S], F32, kind="ExternalInput").ap()
    sinS = nc.dram_tensor("sinS", [128, S], F32, kind="ExternalInput").ap()
    tri = nc.dram_tensor("tri", [128, 128], BF16, kind="ExternalInput").ap()
    eye = nc.dram_tensor("eye", [128, 128], BF16, kind="ExternalInput").ap()
    out = nc.dram_tensor("out", [S, DQ], F32, kind="ExternalOutput").ap()

    hid_r = hid_sw.rearrange("p (c s) -> p c s", c=KC)

    with tile.TileContext(nc) as tc:
        with (
            tc.tile_pool(name="persist", bufs=1) as pp,
            tc.tile_pool(name="dram", bufs=1, space="DRAM") as dramp,
        ):
            # ---- persistent SBUF state
            qTr = [
                [
                    pp.tile([128, 2 * WW], BF16, tag=f"qTr{h}_{nq}", name=f"qTr{h}_{nq}")
                    for nq in range(4)
                ]
                for h in range(HQ)
            ]
            kTr = [
                pp.tile([128, 2 * WW], BF16, tag=f"kTr{nq}", name=f"kTr{nq}")
                for nq in range(4)
            ]
            vNat = [
                pp.tile([128, 4 * VB], BF16, tag=f"vNat{nq}", name=f"vNat{nq}")
                for nq in range(4)
            ]
            tri_sb = pp.tile([128, 128], BF16, tag="tri", name="tri_sb")
            eye_sb = pp.tile([128, 128], BF16, tag="eye", name="eye_sb")
            wq_sb = pp.tile([128, KC * DQ], BF16, tag="wq", name="wq_sb")
            wk_sb = pp.tile([128, KC * HD], BF16, tag="wk", name="wk_sb")
            wv_sb = pp.tile([128, KC * HD], BF16, tag="wv", name="wv_sb")
            wo_sb = pp.tile([128, KC * DQ], BF16, tag="wo", name="wo_sb")
            cos_all = pp.tile([128, S], F32, tag="cos", name="cos_all")
            sin_all = pp.tile([128, S], F32, tag="sin", name="sin_all")

            # ---- DRAM collective buffers: [hd, q] layout (at transposed
            # on the PE before the AllGather)
            ag_ins = [
                dramp.tile([DQ, 256], BF16, tag=f"agin{c}", name=f"agin{c}")
                for c in range(NPAIR)
            ]
            ag_outs = [
                dramp.tile(
                    [H, 256], BF16, tag=f"agout{c}", name=f"agout{c}",
                    addr_space="Shared",
                )
                for c in range(NPAIR)
            ]

            # Warm up the CC stream immediately (first collective pays a
            # ~88us barrier; keep the stream busy until the first real AG).
            warm_in = dramp.tile([128, 8], BF16, tag="win", name="warm_in")
            warm_sb = pp.tile([128, 8], BF16, tag="wsb", name="warm_sb")
            nc.vector.memset(warm_sb[:], 0.0)
            nc.sync.dma_start(warm_in[:], warm_sb[:])
            warm_outs = [
                dramp.tile(
                    [N_CORES * 128, 8], BF16, tag=f"wout{w}", name=f"warm_out{w}",
                    addr_space="Shared",
                )
                for w in range(2)
            ]
            for w in range(2):
                nc.gpsimd.collective_compute(
                    "AllGather",
                    mybir.AluOpType.bypass,
                    replica_groups=[list(range(N_CORES))],
                    ins=[warm_in.opt()],
                    outs=[warm_outs[w].opt()],
                )

            nc.sync.dma_start(eye_sb[:], eye[:])
            nc.sync.dma_start(tri_sb[:], tri[:])
            ones_sb = pp.tile([128, 128], BF16, tag="ones", name="ones_sb")
            nc.vector.memset(ones_sb[:], 1.0)

            with (
                tc.tile_pool(name="hidp", bufs=2) as hidp,
                tc.tile_pool(name="small", bufs=2) as sp,
                tc.tile_pool(name="ep", bufs=3) as ep,
                tc.tile_pool(name="asb", bufs=4) as asb,
                tc.tile_pool(name="agp", bufs=4) as agp,
                tc.tile_pool(name="evp", bufs=2) as evp,
                tc.tile_pool(name="pjp", bufs=1, space="PSUM") as pjp,
                tc.tile_pool(name="sgp", bufs=2, space="PSUM") as sgp,
                tc.tile_pool(name="onp", bufs=1, space="PSUM") as onp,
                tc.tile_pool(name="opp", bufs=1, space="PSUM") as opp,
            ):
                # ---------- staging: weights + window-0 hid interleaved
                hid_tiles = {}
                h0 = hidp.tile([128, KC * WW], BF16, tag="hid", name="hid_w0")
                hid_tiles[0] = h0
                h0_r = h0.rearrange("p (c s) -> p c s", c=KC)
                bounds = [0, 1, 2, 4, 6, 8] + list(range(12, KC + 1, 4))
                pieces = list(zip(bounds[:-1], bounds[1:]))
                def stage_piece(a, b):
                    nc.sync.dma_start(
                        wq_sb[:, a * DQ : b * DQ], wq_sw[:, a * DQ : b * DQ]
                    )
                    nc.sync.dma_start(
                        wk_sb[:, a * HD : b * HD], wk_sw[:, a * HD : b * HD]
                    )
                    nc.sync.dma_start(
                        wv_sb[:, a * HD : b * HD], wv_sw[:, a * HD : b * HD]
                    )
                    nc.sync.dma_start(h0_r[:, a:b, :], hid_r[:, a:b, 0:WW])

                for (a, b) in pieces:
                    if a >= 16:
                        break
                    stage_piece(a, b)
                # window-1 hid early, ahead of the second half of the
                # weights, so window 1 never waits on it
                h1 = hidp.tile([128, KC * WW], BF16, tag="hid", name="hid_w1")
                h1_r = h1.rearrange("p (c s) -> p c s", c=KC)
                nc.sync.dma_start(h1_r[:], hid_r[:, :, WW : 2 * WW])
                hid_tiles[1] = h1
                for (a, b) in pieces:
                    if a >= 16:
                        stage_piece(a, b)
                nc.sync.dma_start(cos_all[:, 0 : S // 2], cosF[:, 0 : S // 2])
                nc.sync.dma_start(sin_all[:, 0 : S // 2], sinS[:, 0 : S // 2])

                il = _IL()
                ag_issued = [False] * NPAIR

                def issue_ag(p):
                    nc.gpsimd.collective_compute(
                        "AllGather",
                        mybir.AluOpType.bypass,
                        replica_groups=[list(range(N_CORES))],
                        ins=[ag_ins[p].opt()],
                        outs=[ag_outs[p].opt()],
                    )
                    ag_issued[p] = True

                def ag_gen(p):
                    # interleaver sentinel: issues AG[p] when the FIFO
                    # reaches it (i.e. right after pair p's units finish)
                    issue_ag(p)
                    return
                    yield

                def attn_unit(p, h):
                    """Generator: attention for pair p, head h, in quanta.

                    All key blocks are computed 256 wide (both q subblocks);
                    invalid halves are zeroed after exp.  PV runs transposed
                    (lhsT = V block) so the output lands directly in the
                    [d, q] AllGather layout -- no PE output transposes.  The
                    softmax denominators come from 1-column-stationary
                    matmuls accumulated in the right half of the PV bank
                    (start=True of the first PV matmul clears the whole
                    bank's has_written bits, so the rowsum chain uses
                    start=False throughout).
                    """
                    q0 = p * WW
                    qq = q0 // 512
                    qbase = q0 - qq * 512
                    blocks = _pair_blocks(p)
                    nb = len(blocks)
                    e_t = ep.tile([128, 2560], BF16, tag="e", name="e_t")

                    for (g, g_end) in _groups(blocks):
                        s_grp = sgp.tile([128, 512], F32, tag="sg", name="s_grp")
                        for bi in range(g, g_end):
                            j, l, r = blocks[bi]
                            # half-valid blocks only compute the valid half;
                            # the stale other half is zeroed after exp
                            qo = 0 if l else 128
                            qw = 256 if (l and r) else 128
                            nc.tensor.matmul(
                                s_grp[
                                    :,
                                    (bi - g) * 256 + qo : (bi - g) * 256 + qo + qw,
                                ],
                                kTr[j // 4][:, (j % 4) * 128 : (j % 4 + 1) * 128],
                                qTr[h][qq][:, qbase + qo : qbase + qo + qw],
                                start=True,
                                stop=True,
                            )
                        gw = (g_end - g) * 256
                        nc.scalar.activation(
                            e_t[:, g * 256 : g * 256 + gw],
                            s_grp[:, 0:gw],
                            mybir.ActivationFunctionType.Exp,
                            scale=SCALE,
                        )
                        for bi in range(g, g_end):
                            j, l, r = blocks[bi]
                            o = bi * 256
                            if not l:
                                nc.vector.memset(e_t[:, o : o + 128], 0.0)
                            elif j == 2 * p:
                                nc.vector.tensor_mul(
                                    e_t[:, o : o + 128],
                                    e_t[:, o : o + 128],
                                    tri_sb[:],
                                )
                            if not r:
                                nc.vector.memset(e_t[:, o + 128 : o + 256], 0.0)
                            elif j == 2 * p + 1:
                                nc.vector.tensor_mul(
                                    e_t[:, o + 128 : o + 256],
                                    e_t[:, o + 128 : o + 256],
                                    tri_sb[:],
                                )
                        yield

                    # PV (transposed): o_both[:, 0:256] = sum_b V_b.T @ e_b
                    # ([d, q]).  Denominators: DVE folds the blocks
                    # (e_sum[k,q] = sum_b e_b[k,q]), then ONE all-ones matmul
                    # does the k-reduction into o_both[:, 256:512] (broadcast
                    # across partitions by the all-ones lhsT) -- saving ~9
                    # rowsum matmuls per unit on the throttled PE.
                    o_both = onp.tile([128, 512], F32, tag="on", name="o_both")
                    half = nb // 2
                    es_parts = []
                    for s, e in ((0, half), (half, nb)):
                        for bi in range(s, e):
                            j, _, _ = blocks[bi]
                            nc.tensor.matmul(
                                o_both[:, 0:256],
                                vNat[j // 4][:, (j % 4) * VB : (j % 4) * VB + 128],
                                e_t[:, bi * 256 : (bi + 1) * 256],
                                start=(bi == 0),
                                stop=(bi == nb - 1),
                            )
                        # partial denominator folds (split so no single DVE
                        # op exceeds ~1.7us and blocks RoPE ops in the FIFO)
                        es = asb.tile(
                            [128, 256], F32, tag=f"es{s != 0}",
                            name="es", bufs=1,
                        )
                        if e - s > 1:
                            nc.vector.tensor_reduce(
                                es[:],
                                e_t[:, s * 256 : e * 256].rearrange(
                                    "p (b q) -> p q b", b=e - s
                                ),
                                axis=mybir.AxisListType.X,
                                op=mybir.AluOpType.add,
                            )
                        else:
                            nc.vector.tensor_copy(es[:], e_t[:, s * 256 : e * 256])
                        es_parts.append(es)
                        yield

                    e_sum = asb.tile(
                        [128, 256], BF16, tag="esb", name="e_sum", bufs=2
                    )
                    nc.vector.tensor_add(e_sum[:], es_parts[0][:], es_parts[1][:])
                    nc.tensor.matmul(
                        o_both[:, 256:512],
                        ones_sb[:],
                        e_sum[:],
                        start=False,
                        stop=True,
                        skip_group_check=True,
                    )
                    rbr = asb.tile([128, 256], F32, tag="rbr", name="rbr", bufs=2)
                    nc.vector.reciprocal_approx_fast(rbr[:], o_both[:, 256:512])
                    at_cT = asb.tile(
                        [128, 256], BF16, tag="at", name="at_cT", bufs=8
                    )
                    nc.vector.tensor_mul(at_cT[:], o_both[:, 0:256], rbr[:])
                    nc.sync.dma_start(
                        ag_ins[p][h * 128 : (h + 1) * 128, :], at_cT[:]
                    )
                    yield

                def unit_quanta(p):
                    return len(_groups(_pair_blocks(p))) + 4

                # ---------- RoPE per window pass
                def rope_pass(w, srcs):
                    """srcs: list of (idx, psum_tile, dstT_slice)."""
                    cw = slice(w * WW, (w + 1) * WW)
                    raws = []
                    for idx, ps_x, dstT in srcs:
                        raw = sp.tile(
                            [128, WW], BF16, tag=f"raw{idx}", name=f"raw{idx}"
                        )
                        nc.vector.tensor_copy(raw[:], ps_x[:])  # sole PSUM read
                        raws.append(raw)
                    for (idx, ps_x, dstT), raw in zip(srcs, raws):
                        swp = sp.tile(
                            [128, WW], BF16, tag=f"swp{idx}", name=f"swp{idx}"
                        )
                        nc.sync.dma_start(swp[0:64, :], raw[64:128, :])
                        nc.sync.dma_start(swp[64:128, :], raw[0:64, :])
                        t1 = sp.tile([128, WW], BF16, tag=f"t1_{idx}", name=f"t1_{idx}")
                        nc.vector.tensor_mul(t1[:], raw[:], cos_all[:, cw])
                        t2 = sp.tile([128, WW], BF16, tag="t2", name="t2", bufs=4)
                        nc.vector.tensor_mul(t2[:], swp[:], sin_all[:, cw])
                        nc.vector.tensor_add(dstT[:], t1[:], t2[:])

                # ---------- window loop
                deferred_pe = []  # V transposes deferred into next window

                for w in range(NPAIR):
                    if 1 <= w and w + 1 < NPAIR:
                        h2 = hidp.tile(
                            [128, KC * WW], BF16, tag="hid", name=f"hid_w{w+1}"
                        )
                        h2_r = h2.rearrange("p (c s) -> p c s", c=KC)
                        nc.sync.dma_start(
                            h2_r[:], hid_r[:, :, (w + 1) * WW : (w + 2) * WW]
                        )
                        hid_tiles[w + 1] = h2
                    if w == 2:
                        nc.sync.dma_start(
                            cos_all[:, S // 2 :], cosF[:, S // 2 :]
                        )
                        nc.sync.dma_start(
                            sin_all[:, S // 2 :], sinS[:, S // 2 :]
                        )
                    if 3 <= w < 7:
                        # trickle wo in quarters during windows 3-6 (needed
                        # only by the o_proj tail; keeps it off the
                        # congested startup HBM window)
                        q = KC * DQ // 4
                        nc.sync.dma_start(
                            wo_sb[:, (w - 3) * q : (w - 2) * q],
                            wo_sw[:, (w - 3) * q : (w - 2) * q],
                        )

                    hid_c = hid_tiles.pop(w)
                    npend = 4 * unit_quanta(w - 1) if w >= 1 else 0
                    # pump positions among the 64 chunk-passes (start late
                    # enough that RoPE of window w-1 has landed)
                    positions = {}
                    if npend:
                        span = 64 - 6
                        for k in range(npend):
                            pos = 5 + (k * span) // npend
                            positions[pos] = positions.get(pos, 0) + 1

                    qq, half = w // 2, w % 2
                    cp = 0
                    for pas in range(2):
                        ha, hb = (0, 1) if pas == 0 else (2, 3)
                        ps_a = pjp.tile([128, WW], F32, tag="pa0", name="ps_a")
                        ps_b = pjp.tile([128, WW], F32, tag="pa1", name="ps_b")
                        ps_kv = pjp.tile([128, WW], F32, tag="pk", name="ps_kv")
                        wkv_sb = wk_sb if pas == 0 else wv_sb
                        for c in range(KC):
                            st, sp_ = (c == 0), (c == KC - 1)
                            hs = hid_c[:, c * WW : (c + 1) * WW]
                            nc.tensor.matmul(
                                ps_a[:],
                                wq_sb[:, c * DQ + ha * HD : c * DQ + (ha + 1) * HD],
                                hs, start=st, stop=sp_,
                            )
                            nc.tensor.matmul(
                                ps_b[:],
                                wq_sb[:, c * DQ + hb * HD : c * DQ + (hb + 1) * HD],
                                hs, start=st, stop=sp_,
                            )
                            nc.tensor.matmul(
                                ps_kv[:], wkv_sb[:, c * HD : (c + 1) * HD], hs,
                                start=st, stop=sp_,
                            )
                            if cp == 2 and deferred_pe:
                                for fn in deferred_pe:
                                    fn()
                                deferred_pe = []
                            for _ in range(positions.get(cp, 0)):
                                il.pump()
                            cp += 1

                        dsl = slice(half * WW, (half + 1) * WW)
                        if pas == 0:
                            srcs = [
                                (2, ps_kv, kTr[qq][:, dsl]),
                                (0, ps_a, qTr[0][qq][:, dsl]),
                                (1, ps_b, qTr[1][qq][:, dsl]),
                            ]
                            rope_pass(w, srcs)
                        else:
                            srcs = [
                                (3, ps_a, qTr[2][qq][:, dsl]),
                                (4, ps_b, qTr[3][qq][:, dsl]),
                            ]
                            rope_pass(w, srcs)
                            # V: evacuate + 2 PE transposes (deferred into
                            # the next window so the PE never waits here)
                            vT_q = sp.tile([128, WW], BF16, tag="vT", name="vT_q")
                            nc.vector.tensor_copy(vT_q[:], ps_kv[:])

                            def v_tr(w=w, vT_q=vT_q, qq=qq, half=half):
                                for b_ in range(2):
                                    tr = sgp.tile(
                                        [128, 128], BF16, tag="sg", name="tr"
                                    )
                                    nc.tensor.transpose(
                                        tr[:],
                                        vT_q[:, b_ * 128 : (b_ + 1) * 128],
                                        eye_sb[:],
                                    )
                                    blk = 2 * half + b_
                                    nc.vector.tensor_copy(
                                        vNat[qq][:, blk * VB : blk * VB + 128], tr[:]
                                    )

                            deferred_pe.append(v_tr)

                    il.drain()
                    if w + 1 < NPAIR:
                        for h in range(HQ):
                            il.add(attn_unit(w, h))
                        il.add(ag_gen(w))

                # ---------- tail: pair-7 attention + o_proj
                for fn in deferred_pe:
                    fn()
                deferred_pe = []
                for h in range(HQ):
                    il.add(attn_unit(NPAIR - 1, h))
                il.add(ag_gen(NPAIR - 1))

                def ensure_ag(p):
                    while not ag_issued[p]:
                        if not il.pump():
                            raise RuntimeError(f"AG {p} never issued")

                ag_sbs = {}

                def oproj_dma(p, r):
                    # One batched load per slice on the GpSimd (SWDGE)
                    # queue: nothing latency-critical shares that queue, so
                    # its waits on AllGather completion can't block
                    # exp/RoPE/hid DMAs, and batching keeps the Q7 issue
                    # cost (~0.6us per dma_start) off the tail critical
                    # path.
                    t = agp.tile([128, 4 * 256], BF16, tag="ag", name="ag_sb", bufs=4)
                    t_r = t.rearrange("p (c q) -> p c q", c=4)
                    src = ag_outs[p][512 * r : 512 * (r + 1), :].rearrange(
                        "(c p) q -> p c q", c=4
                    )
                    nc.gpsimd.dma_start(t_r, src)
                    ag_sbs[(p, r)] = t

                def oproj_mm(p, r, ps01):
                    # 4 N=256 accumulation chains (2 col-halves per bank):
                    # N=256 LDW+MM chains sustain ~110-140ns/MM on this HW
                    # where N=512 pairs measure ~283ns.
                    t = ag_sbs.pop((p, r))
                    for c2 in range(4):
                        c = 4 * r + c2
                        for sb in range(2):
                            for ch in range(2):
                                nc.tensor.matmul(
                                    ps01[sb][:, ch * 256 : (ch + 1) * 256],
                                    t[:, c2 * 256 + sb * 128 : c2 * 256 + (sb + 1) * 128],
                                    wo_sb[:, c * DQ + ch * 256 : c * DQ + (ch + 1) * 256],
                                    start=(c == 0 and ch == 0),
                                    stop=(c == KC - 1),
                                    skip_group_check=(ch == 1),
                                )

                def oproj_finish(p, ps01):
                    q0 = p * 256
                    for sb in range(2):
                        ev = evp.tile([128, DQ], F32, tag="ev", name="ev")
                        nc.vector.tensor_copy(ev[:], ps01[sb][:])
                        nc.sync.dma_start(
                            out[q0 + sb * 128 : q0 + (sb + 1) * 128, :], ev[:]
                        )

                seq = [(p, r) for p in range(NPAIR) for r in range(N_CORES)]
                ensure_ag(seq[0][0])
                oproj_dma(*seq[0])
                oproj_dma(*seq[1])
                ps_map = {}
                for k, (p, r) in enumerate(seq):
                    if k + 2 < len(seq):
                        ensure_ag(seq[k + 2][0])
                        oproj_dma(*seq[k + 2])
                    if r == 0:
                        ps_map[p] = [
                            opp.tile([128, DQ], F32, tag=f"op{sb}", name=f"op{sb}")
                            for sb in range(2)
                        ]
                    oproj_mm(p, r, ps_map[p])
                    il.pump()
                    il.pump()
                    if r == N_CORES - 1:
                        oproj_finish(p, ps_map.pop(p))
                il.drain()
                assert all(ag_issued)

    nc.compile()
    return nc


@functools.lru_cache(maxsize=1)
def _cached_nc():
    return build_nc()


def _tables():
    pos = np.arange(S, dtype=np.float64)
    inv = 1.0 / (ROPE_BASE ** (np.arange(0, HD, 2, dtype=np.float64) / HD))  # [64]
    f = inv[:, None] * pos[None, :]                   # [64, S]
    cos = np.cos(f).astype(np.float32)
    sin = np.sin(f).astype(np.float32)
    cosF = np.concatenate([cos, cos], axis=0)         # [128, S]
    sinS = np.concatenate([-sin, sin], axis=0)        # [128, S]
    k_idx = np.arange(128)[:, None]
    q_idx = np.arange(128)[None, :]
    tri = (k_idx <= q_idx).astype(np.float32)         # [k, q] causal in-block
    return cosF, sinS, tri


def _swz(w: np.ndarray, bf16) -> np.ndarray:
    """[KC*128, W] -> chunk-major [128, KC*W] bf16."""
    kc, w_ = w.shape[0] // 128, w.shape[1]
    return np.ascontiguousarray(
        w.reshape(kc, 128, w_).transpose(1, 0, 2).reshape(128, kc * w_)
    ).astype(bf16)


def _run(hidden_states, wq, wk, wv, wo, **run_kwargs):
    nc = _cached_nc()
    bf16 = mybir.dt.np(BF16)
    # hid_sw[p, c*S + s] = hidden[s, c*128 + p]
    hid2 = np.asarray(hidden_states, dtype=np.float32).reshape(S, H)
    hid_sw = np.ascontiguousarray(
        hid2.reshape(S, KC, 128).transpose(2, 1, 0).reshape(128, KC * S)
    ).astype(bf16)
    cosF, sinS, tri = _tables()
    in_maps = []
    for c in range(N_CORES):
        in_maps.append(
            {
                "hid_sw": hid_sw,
                "wq_sw": _swz(wq[:, c * DQ : (c + 1) * DQ], bf16),
                "wk_sw": _swz(wk[:, c * HD : (c + 1) * HD], bf16),
                "wv_sw": _swz(wv[:, c * HD : (c + 1) * HD], bf16),
                "wo_sw": _swz(wo[:, c * DQ : (c + 1) * DQ], bf16),
                "cosF": cosF,
                "sinS": sinS,
                "tri": tri.astype(bf16),
                "eye": np.eye(128, dtype=np.float32).astype(bf16),
            }
        )
    res = run_bass_kernel_spmd(
        nc, in_maps, core_ids=list(range(N_CORES)), **run_kwargs
    )
    full = np.concatenate(
        [res.results[r]["out"] for r in range(N_CORES)], axis=1
    )
    return full.reshape(B, S, H).astype(np.float32), res


def kernel(hidden_states, wq, wk, wv, wo):
    out, _ = _run(hidden_states, wq, wk, wv, wo)
    return out
